# revision 20
# baseline (speedup 1.0000x reference)
"""Bass/Trainium2 kernel for nn_DynamicEdgeWeights.

Math (B=4, S=512, D=128, H=8):
    a = x @ w1[:D]; c = x @ w1[D:]
    h[b,i,j,:] = relu(a[b,i,:] + c[b,j,:] + b1)
    out[b,h,i,j] = sigmoid(sum_d h[b,i,j,d] * w2[d,h] + b2[h])

Device strategy (per core; 8 cores, core k -> batch k//2, i-rows [(k%2)*256, +256)):
  - cT[d, j] = (x[b] @ w1c).T and aT[d, i] = (x[b] @ w1a).T + b1 via two PE
    matmuls on pre-transposed x (host passes x[b].T).
  - per query row i: one fused relu(cT + aT[:, i]) producing h_i [128d, 512j]
    (DVE tensor_scalar add+max, or ACT activation Relu with per-partition bias).
  - second matmul uses "comb" weights: 16 query rows share one PSUM bank.
    comb_g [128, 128] has w2[:, h] in column h*16+g, zeros elsewhere; 16
    accumulating matmuls put e-pre for (16 i x 8 h) on 128 PSUM partitions.
  - one full-width sigmoid (ACT, bias=b2 broadcast) -> SBUF -> one 256 KiB DMA
    straight into out[b, :, i0+t*16 : i0+(t+1)*16, :].
"""

import sys

if "/opt/trn_rl_repo" not in sys.path:
    sys.path.insert(0, "/opt/trn_rl_repo")

import numpy as np
import ml_dtypes  # noqa: F401  (registers bfloat16 dtype)

import concourse.bass as bass  # noqa: F401  (registers types)
import concourse.mybir as mybir
from concourse import bacc
from concourse.bass_utils import run_bass_kernel_spmd
from concourse.tile import TileContext

B, S, D, H = 4, 512, 128, 8
N_CORES = 8
I_PER_CORE = (B * S) // N_CORES  # 256
G = 16  # query rows packed per PSUM bank
T = I_PER_CORE // G  # 16 groups per core
N_ACT = 3  # of every 16 h-gen ops, this many go to ScalarE (rest DVE)

F32 = mybir.dt.float32
F16 = mybir.dt.float16  # h-path dtype: full PE rate (fp32 streams at 1/4 rate)
F16_NP = "float16"

_CACHE: dict = {}


def _build_nc(loop_iters: int = 1, dt_h=F16, n_act=N_ACT, h_bufs=6, o_bufs=3, mm_bufs=2, staggered=False, diag=None):
    """Build the single-core Bass program (identical across the 8 cores).

    loop_iters > 1 wraps the whole compute in an on-device For_i repeat —
    used only for steady-state timing (one dispatch, N executions).
    """
    nc = bacc.Bacc(
        "TRN2",
        target_bir_lowering=False,
        debug=False,
        enable_asserts=False,
        num_devices=N_CORES,
    )

    xTj_d = nc.dram_tensor("xTj", (D, S), F32, kind="ExternalInput").ap()
    xTi_d = nc.dram_tensor("xTi", (D, I_PER_CORE), F32, kind="ExternalInput").ap()
    w1a_d = nc.dram_tensor("w1a", (D, D), F32, kind="ExternalInput").ap()
    w1c_d = nc.dram_tensor("w1c", (D, D), F32, kind="ExternalInput").ap()
    b1_d = nc.dram_tensor("b1c", (D, 1), F32, kind="ExternalInput").ap()
    comb_d = nc.dram_tensor("comb", (D, G * D), dt_h, kind="ExternalInput").ap()
    b2v_d = nc.dram_tensor("b2v", (D, 1), F32, kind="ExternalInput").ap()
    out_d = nc.dram_tensor("out", (H, I_PER_CORE, S), F32, kind="ExternalOutput").ap()

    relu = mybir.ActivationFunctionType.Relu
    sigmoid = mybir.ActivationFunctionType.Sigmoid
    add = mybir.AluOpType.add
    amax = mybir.AluOpType.max

    import contextlib

    with TileContext(nc) as tc:
        with (
            tc.tile_pool(name="const", bufs=1) as cpool,
            tc.tile_pool(name="h", bufs=h_bufs) as hpool,
            tc.tile_pool(name="o", bufs=o_bufs) as opool,
            tc.tile_pool(name="mm", bufs=mm_bufs, space="PSUM") as mmpool,
            (
                tc.For_i(
                    0,
                    loop_iters,
                    1,
                    hint_engines=(
                        mybir.EngineType.PE,
                        mybir.EngineType.DVE,
                        mybir.EngineType.Activation,
                        mybir.EngineType.SP,
                    ),
                    staggered_reset=staggered,
                )
                if loop_iters > 1
                else contextlib.nullcontext()
            ),
        ):
            xj_sb = cpool.tile([D, S], F32)
            nc.sync.dma_start(out=xj_sb, in_=xTj_d)
            xi_sb = cpool.tile([D, I_PER_CORE], F32)
            nc.sync.dma_start(out=xi_sb, in_=xTi_d)
            w1a_sb = cpool.tile([D, D], F32)
            nc.sync.dma_start(out=w1a_sb, in_=w1a_d)
            w1c_sb = cpool.tile([D, D], F32)
            nc.sync.dma_start(out=w1c_sb, in_=w1c_d)
            comb_sb = cpool.tile([D, G * D], dt_h)
            nc.scalar.dma_start(out=comb_sb[:, : G * D // 2], in_=comb_d[:, : G * D // 2])
            nc.scalar.dma_start(out=comb_sb[:, G * D // 2 :], in_=comb_d[:, G * D // 2 :])
            b1_sb = cpool.tile([D, 1], F32)
            nc.sync.dma_start(out=b1_sb, in_=b1_d)
            b2v_sb = cpool.tile([D, 1], F32)
            nc.sync.dma_start(out=b2v_sb, in_=b2v_d)

            # precompute borrows one pair-slot from the matmul psum pool:
            # cT in the first bank-half, aT in the second
            pre_ps = mmpool.tile([D, 2 * S], F32, tag="ps2")
            # cT[d_out, j] = sum_k w1c[k, d_out] * xT[k, j]
            nc.tensor.matmul(pre_ps[:, :S], w1c_sb, xj_sb, start=True, stop=True)
            cT_sb = cpool.tile([D, S], dt_h)
            nc.vector.tensor_copy(cT_sb, pre_ps[:, :S])

            # aT[d_out, i] = sum_k w1a[k, d_out] * xT[k, i]  (+ b1 per partition)
            nc.tensor.matmul(
                pre_ps[:, S : S + I_PER_CORE], w1a_sb, xi_sb, start=True, stop=True
            )
            at_sb = cpool.tile([D, I_PER_CORE], F32)
            nc.vector.tensor_scalar_add(at_sb, pre_ps[:, S : S + I_PER_CORE], b1_sb)

            def drain(t, ps2):
                # sigmoid + store for a finished pair of groups (t, t+1);
                # emitted one pair late so ACT's (stalling) sigmoid sits
                # behind the next pair's h-gen ops in ACT program order.
                o_sb = opool.tile([D, 2 * S], F32)
                nc.scalar.activation(o_sb, ps2, sigmoid, bias=b2v_sb)
                # partition p = h*16+g  ->  out[h, (t+u)*16+g, :]; split
                # stores across both HWDGE queues (SP h 0-3, ACT h 4-7)
                half = D // 2
                for u in range(2):
                    dst = out_d[:, (t + u) * G : (t + u + 1) * G, :]
                    src = o_sb[:, u * S : (u + 1) * S]
                    nc.sync.dma_start(out=dst[: H // 2], in_=src[:half])
                    nc.scalar.dma_start(out=dst[H // 2 :], in_=src[half:])

            if diag == "pe":
                # PE-pure stream: one static h tile, full matmul schedule
                h_static = cpool.tile([D, 2 * S], dt_h)
                nc.vector.tensor_scalar(h_static, cT_sb.rearrange("p s -> p s"), 0.0, None, add, None) if False else None
                nc.vector.tensor_copy(h_static[:, :S], cT_sb)
                nc.vector.tensor_copy(h_static[:, S:], cT_sb)
                for t in range(0, T, 2):
                    ps2 = mmpool.tile([D, 2 * S], F32, tag="ps2")
                    for g in range(G):
                        for u in range(2):
                            nc.tensor.matmul(
                                ps2[:, u * S : (u + 1) * S],
                                comb_sb[:, g * D : (g + 1) * D],
                                h_static[:, u * S : (u + 1) * S],
                                start=(g == 0),
                                stop=(g == G - 1),
                            )
                    o_sb = opool.tile([D, 2 * S], F32)
                    nc.scalar.activation(o_sb, ps2, sigmoid, bias=b2v_sb)
                    half = D // 2
                    for u in range(2):
                        dst = out_d[:, (t + u) * G : (t + u + 1) * G, :]
                        src = o_sb[:, u * S : (u + 1) * S]
                        nc.sync.dma_start(out=dst[: H // 2], in_=src[:half])
                        nc.scalar.dma_start(out=dst[H // 2 :], in_=src[half:])
            elif diag == "dve":
                # DVE-pure stream: all h-gen ops, no matmul/sigmoid; dump one
                # h tile to out to keep outputs written
                for t in range(0, T, 2):
                    for g in range(G):
                        h2 = hpool.tile([D, 2 * S], dt_h)
                        for u in range(2):
                            i_loc = (t + u) * G + g
                            a_col = at_sb[:, i_loc : i_loc + 1]
                            dst = h2[:, u * S : (u + 1) * S]
                            nc.vector.tensor_scalar(dst, cT_sb, a_col, 0.0, add, amax)
                    o_sb = opool.tile([D, 2 * S], F32)
                    nc.vector.tensor_copy(o_sb, h2)
                    half = D // 2
                    for u in range(2):
                        dst = out_d[:, (t + u) * G : (t + u + 1) * G, :]
                        src = o_sb[:, u * S : (u + 1) * S]
                        nc.sync.dma_start(out=dst[: H // 2], in_=src[:half])
                        nc.scalar.dma_start(out=dst[H // 2 :], in_=src[half:])
            else:
                pending = None  # (t, psum tile) awaiting sigmoid+store
                for t in range(0, T, 2):
                    # two groups (t, t+1) share one 2-bank PSUM tile: matmul g
                    # covers j 0..511 for row t*16+g and j 512..1023 for row
                    # (t+1)*16+g with the same comb_g weights.
                    ps2 = mmpool.tile([D, 2 * S], F32, tag="ps2")
                    for g in range(G):
                        h2 = hpool.tile([D, 2 * S], dt_h)
                        for u in range(2):
                            i_loc = (t + u) * G + g
                            a_col = at_sb[:, i_loc : i_loc + 1]
                            dst = h2[:, u * S : (u + 1) * S]
                            if g < n_act:
                                nc.scalar.activation(dst, cT_sb, relu, bias=a_col)
                            else:
                                nc.vector.tensor_scalar(dst, cT_sb, a_col, 0.0, add, amax)
                        for u in range(2):
                            nc.tensor.matmul(
                                ps2[:, u * S : (u + 1) * S],
                                comb_sb[:, g * D : (g + 1) * D],
                                h2[:, u * S : (u + 1) * S],
                                start=(g == 0),
                                stop=(g == G - 1),
                            )
                        if g == 1 and pending is not None:
                            drain(*pending)
                            pending = None
                    pending = (t, ps2)
                drain(*pending)

    nc.compile()
    return nc


def _host_prep(node_features, w1, b1, w2, b2):
    """Shared (per-core-replicated) small tensors + per-core input maps."""
    w1a = np.ascontiguousarray(w1[:D])  # [D, D] == lhsT for aT
    w1c = np.ascontiguousarray(w1[D:])  # [D, D] == lhsT for cT
    b1c = np.ascontiguousarray(b1.reshape(D, 1))
    comb = np.zeros((D, G, D), np.float32)
    cols = np.arange(H) * G
    for g in range(G):
        comb[:, g, cols + g] = w2
    comb = np.ascontiguousarray(comb.reshape(D, G * D).astype(F16_NP))
    b2v = np.ascontiguousarray(np.repeat(b2, G).reshape(D, 1))

    in_maps = []
    for k in range(N_CORES):
        b = k // (N_CORES // B)
        i0 = (k % (N_CORES // B)) * I_PER_CORE
        xT = np.ascontiguousarray(node_features[b].T)  # [D, S]
        in_maps.append(
            {
                "xTj": xT,
                "xTi": np.ascontiguousarray(xT[:, i0 : i0 + I_PER_CORE]),
                "w1a": w1a,
                "w1c": w1c,
                "b1c": b1c,
                "comb": comb,
                "b2v": b2v,
            }
        )
    return in_maps


def _gather(results):
    out = np.empty((B, H, S, S), np.float32)
    for k in range(N_CORES):
        b = k // (N_CORES // B)
        i0 = (k % (N_CORES // B)) * I_PER_CORE
        out[b, :, i0 : i0 + I_PER_CORE, :] = results[k]["out"]
    return out


def _run(in_maps, **kwargs):
    if "nc" not in _CACHE:
        _CACHE["nc"] = _build_nc()
    return run_bass_kernel_spmd(
        _CACHE["nc"], in_maps, core_ids=list(range(N_CORES)), **kwargs
    )


def kernel(node_features, w1, b1, w2, b2):
    node_features = np.asarray(node_features, np.float32)
    w1 = np.asarray(w1, np.float32)
    b1 = np.asarray(b1, np.float32)
    w2 = np.asarray(w2, np.float32)
    b2 = np.asarray(b2, np.float32)
    in_maps = _host_prep(node_features, w1, b1, w2, b2)
    res = _run(in_maps)
    return _gather(res.results)


def kernel_traced(node_features, w1, b1, w2, b2, **trace_kwargs):
    """test.py helper: same as kernel() but with NTFF tracing enabled."""
    in_maps = _host_prep(
        np.asarray(node_features, np.float32),
        np.asarray(w1, np.float32),
        np.asarray(b1, np.float32),
        np.asarray(w2, np.float32),
        np.asarray(b2, np.float32),
    )
    res = _run(in_maps, trace=True, **trace_kwargs)
    return _gather(res.results), res


# revision 21
# speedup vs baseline: 1.2105x; 1.2105x over previous
"""Bass/Trainium2 kernel for nn_DynamicEdgeWeights.

Math (B=4, S=512, D=128, H=8):
    a = x @ w1[:D]; c = x @ w1[D:]
    h[b,i,j,:] = relu(a[b,i,:] + c[b,j,:] + b1)
    out[b,h,i,j] = sigmoid(sum_d h[b,i,j,d] * w2[d,h] + b2[h])

Device strategy (per core; 8 cores, core k -> batch k//2, i-rows [(k%2)*256, +256)):
  - cT[d, j] = (x[b] @ w1c).T and aT[d, i] = (x[b] @ w1a).T + b1 via two PE
    matmuls on pre-transposed x (host passes x[b].T).
  - per query row i: one fused relu(cT + aT[:, i]) producing h_i [128d, 512j]
    (DVE tensor_scalar add+max, or ACT activation Relu with per-partition bias).
  - second matmul uses "comb" weights: 16 query rows share one PSUM bank.
    comb_g [128, 128] has w2[:, h] in column h*16+g, zeros elsewhere; 16
    accumulating matmuls put e-pre for (16 i x 8 h) on 128 PSUM partitions.
  - one full-width sigmoid (ACT, bias=b2 broadcast) -> SBUF -> one 256 KiB DMA
    straight into out[b, :, i0+t*16 : i0+(t+1)*16, :].
"""

import sys

if "/opt/trn_rl_repo" not in sys.path:
    sys.path.insert(0, "/opt/trn_rl_repo")

import numpy as np
import ml_dtypes  # noqa: F401  (registers bfloat16 dtype)

import concourse.bass as bass  # noqa: F401  (registers types)
import concourse.mybir as mybir
from concourse import bacc
from concourse.bass_utils import run_bass_kernel_spmd
from concourse.tile import TileContext

B, S, D, H = 4, 512, 128, 8
N_CORES = 8
I_PER_CORE = (B * S) // N_CORES  # 256
G = 16  # query rows packed per PSUM bank
T = I_PER_CORE // G  # 16 groups per core
N_ACT = 3  # of every 16 h-gen ops, this many go to ScalarE (rest DVE)

F32 = mybir.dt.float32
F16 = mybir.dt.float16  # h-path dtype: full PE rate (fp32 streams at 1/4 rate)
F16_NP = "float16"

_CACHE: dict = {}


def _build_nc(loop_iters: int = 1, dt_h=F16, n_act=N_ACT, h_bufs=6, o_bufs=3, mm_bufs=2, staggered=False, diag=None):
    """Build the single-core Bass program (identical across the 8 cores).

    loop_iters > 1 wraps the whole compute in an on-device For_i repeat —
    used only for steady-state timing (one dispatch, N executions).
    """
    nc = bacc.Bacc(
        "TRN2",
        target_bir_lowering=False,
        debug=False,
        enable_asserts=False,
        num_devices=N_CORES,
    )

    xTj_d = nc.dram_tensor("xTj", (D, S), F32, kind="ExternalInput").ap()
    xTi_d = nc.dram_tensor("xTi", (D, I_PER_CORE), F32, kind="ExternalInput").ap()
    w1a_d = nc.dram_tensor("w1a", (D, D), F32, kind="ExternalInput").ap()
    w1c_d = nc.dram_tensor("w1c", (D, D), F32, kind="ExternalInput").ap()
    b1_d = nc.dram_tensor("b1c", (D, 1), F32, kind="ExternalInput").ap()
    comb_d = nc.dram_tensor("comb", (D, G * D), dt_h, kind="ExternalInput").ap()
    b2v_d = nc.dram_tensor("b2v", (D, 1), F32, kind="ExternalInput").ap()
    out_d = nc.dram_tensor("out", (H, I_PER_CORE, S), F32, kind="ExternalOutput").ap()

    relu = mybir.ActivationFunctionType.Relu
    sigmoid = mybir.ActivationFunctionType.Sigmoid
    add = mybir.AluOpType.add
    amax = mybir.AluOpType.max

    import contextlib

    with TileContext(nc) as tc:
        with (
            tc.tile_pool(name="const", bufs=1) as cpool,
            tc.tile_pool(name="h", bufs=h_bufs) as hpool,
            tc.tile_pool(name="o", bufs=o_bufs) as opool,
            tc.tile_pool(name="mm", bufs=mm_bufs, space="PSUM") as mmpool,
            (
                tc.For_i(
                    0,
                    loop_iters,
                    1,
                    hint_engines=(
                        mybir.EngineType.PE,
                        mybir.EngineType.DVE,
                        mybir.EngineType.Activation,
                        mybir.EngineType.SP,
                    ),
                    staggered_reset=staggered,
                )
                if loop_iters > 1
                else contextlib.nullcontext()
            ),
        ):
            xj_sb = cpool.tile([D, S], F32)
            nc.sync.dma_start(out=xj_sb, in_=xTj_d)
            xi_sb = cpool.tile([D, I_PER_CORE], F32)
            nc.sync.dma_start(out=xi_sb, in_=xTi_d)
            w1a_sb = cpool.tile([D, D], F32)
            nc.sync.dma_start(out=w1a_sb, in_=w1a_d)
            w1c_sb = cpool.tile([D, D], F32)
            nc.sync.dma_start(out=w1c_sb, in_=w1c_d)
            comb_sb = cpool.tile([D, G * D], dt_h)
            nc.scalar.dma_start(out=comb_sb[:, : G * D // 2], in_=comb_d[:, : G * D // 2])
            nc.scalar.dma_start(out=comb_sb[:, G * D // 2 :], in_=comb_d[:, G * D // 2 :])
            b1_sb = cpool.tile([D, 1], F32)
            nc.sync.dma_start(out=b1_sb, in_=b1_d)
            b2v_sb = cpool.tile([D, 1], F32)
            nc.sync.dma_start(out=b2v_sb, in_=b2v_d)

            # precompute borrows one pair-slot from the matmul psum pool:
            # cT in the first bank-half, aT in the second
            pre_ps = mmpool.tile([D, 2 * S], F32, tag="ps2")
            # cT[d_out, j] = sum_k w1c[k, d_out] * xT[k, j]
            nc.tensor.matmul(pre_ps[:, :S], w1c_sb, xj_sb, start=True, stop=True)
            cT_sb = cpool.tile([D, S], dt_h)
            nc.vector.tensor_copy(cT_sb, pre_ps[:, :S])

            # aT[d_out, i] = sum_k w1a[k, d_out] * xT[k, i]  (+ b1 per partition)
            nc.tensor.matmul(
                pre_ps[:, S : S + I_PER_CORE], w1a_sb, xi_sb, start=True, stop=True
            )
            at_sb = cpool.tile([D, I_PER_CORE], F32)
            nc.vector.tensor_scalar_add(at_sb, pre_ps[:, S : S + I_PER_CORE], b1_sb)

            def drain(t, ps2):
                # sigmoid + store for a finished pair of groups (t, t+1);
                # emitted one pair late so ACT's (stalling) sigmoid sits
                # behind the next pair's h-gen ops in ACT program order.
                o_sb = opool.tile([D, 2 * S], F32)
                nc.scalar.activation(o_sb, ps2, sigmoid, bias=b2v_sb)
                # partition p = h*16+g  ->  out[h, (t+u)*16+g, :]; split
                # stores across both HWDGE queues (SP h 0-3, ACT h 4-7)
                half = D // 2
                for u in range(2):
                    dst = out_d[:, (t + u) * G : (t + u + 1) * G, :]
                    src = o_sb[:, u * S : (u + 1) * S]
                    nc.sync.dma_start(out=dst[: H // 2], in_=src[:half])
                    nc.scalar.dma_start(out=dst[H // 2 :], in_=src[half:])

            if diag == "pe":
                # PE-pure stream: one static h tile, full matmul schedule
                h_static = cpool.tile([D, 2 * S], dt_h)
                nc.vector.tensor_scalar(h_static, cT_sb.rearrange("p s -> p s"), 0.0, None, add, None) if False else None
                nc.vector.tensor_copy(h_static[:, :S], cT_sb)
                nc.vector.tensor_copy(h_static[:, S:], cT_sb)
                for t in range(0, T, 2):
                    ps2 = mmpool.tile([D, 2 * S], F32, tag="ps2")
                    for g in range(G):
                        for u in range(2):
                            nc.tensor.matmul(
                                ps2[:, u * S : (u + 1) * S],
                                comb_sb[:, g * D : (g + 1) * D],
                                h_static[:, u * S : (u + 1) * S],
                                start=(g == 0),
                                stop=(g == G - 1),
                            )
                    o_sb = opool.tile([D, 2 * S], F32)
                    nc.scalar.activation(o_sb, ps2, sigmoid, bias=b2v_sb)
                    half = D // 2
                    for u in range(2):
                        dst = out_d[:, (t + u) * G : (t + u + 1) * G, :]
                        src = o_sb[:, u * S : (u + 1) * S]
                        nc.sync.dma_start(out=dst[: H // 2], in_=src[:half])
                        nc.scalar.dma_start(out=dst[H // 2 :], in_=src[half:])
            elif diag == "dve":
                # DVE-pure stream: all h-gen ops, no matmul/sigmoid; dump one
                # h tile to out to keep outputs written
                for t in range(0, T, 2):
                    for g in range(G):
                        h2 = hpool.tile([D, 2 * S], dt_h)
                        for u in range(2):
                            i_loc = (t + u) * G + g
                            a_col = at_sb[:, i_loc : i_loc + 1]
                            dst = h2[:, u * S : (u + 1) * S]
                            nc.vector.tensor_scalar(dst, cT_sb, a_col, 0.0, add, amax)
                    o_sb = opool.tile([D, 2 * S], F32)
                    nc.vector.tensor_copy(o_sb, h2)
                    half = D // 2
                    for u in range(2):
                        dst = out_d[:, (t + u) * G : (t + u + 1) * G, :]
                        src = o_sb[:, u * S : (u + 1) * S]
                        nc.sync.dma_start(out=dst[: H // 2], in_=src[:half])
                        nc.scalar.dma_start(out=dst[H // 2 :], in_=src[half:])
            else:
                pending = None  # (t, psum tile) awaiting sigmoid+store
                for t in range(0, T, 2):
                    # two groups (t, t+1) share one 2-bank PSUM tile: matmul g
                    # covers j 0..511 for row t*16+g and j 512..1023 for row
                    # (t+1)*16+g with the same comb_g weights.
                    ps2 = mmpool.tile([D, 2 * S], F32, tag="ps2")
                    for g in range(G):
                        h2 = hpool.tile([D, 2 * S], dt_h)
                        for u in range(2):
                            i_loc = (t + u) * G + g
                            a_col = at_sb[:, i_loc : i_loc + 1]
                            dst = h2[:, u * S : (u + 1) * S]
                            if g >= G - n_act:
                                nc.scalar.activation(dst, cT_sb, relu, bias=a_col)
                            else:
                                nc.vector.tensor_scalar(dst, cT_sb, a_col, 0.0, add, amax)
                        for u in range(2):
                            nc.tensor.matmul(
                                ps2[:, u * S : (u + 1) * S],
                                comb_sb[:, g * D : (g + 1) * D],
                                h2[:, u * S : (u + 1) * S],
                                start=(g == 0),
                                stop=(g == G - 1),
                            )
                        if g == 1 and pending is not None:
                            drain(*pending)
                            pending = None
                    pending = (t, ps2)
                drain(*pending)

    nc.compile()
    return nc


def _host_prep(node_features, w1, b1, w2, b2):
    """Shared (per-core-replicated) small tensors + per-core input maps."""
    w1a = np.ascontiguousarray(w1[:D])  # [D, D] == lhsT for aT
    w1c = np.ascontiguousarray(w1[D:])  # [D, D] == lhsT for cT
    b1c = np.ascontiguousarray(b1.reshape(D, 1))
    comb = np.zeros((D, G, D), np.float32)
    cols = np.arange(H) * G
    for g in range(G):
        comb[:, g, cols + g] = w2
    comb = np.ascontiguousarray(comb.reshape(D, G * D).astype(F16_NP))
    b2v = np.ascontiguousarray(np.repeat(b2, G).reshape(D, 1))

    in_maps = []
    for k in range(N_CORES):
        b = k // (N_CORES // B)
        i0 = (k % (N_CORES // B)) * I_PER_CORE
        xT = np.ascontiguousarray(node_features[b].T)  # [D, S]
        in_maps.append(
            {
                "xTj": xT,
                "xTi": np.ascontiguousarray(xT[:, i0 : i0 + I_PER_CORE]),
                "w1a": w1a,
                "w1c": w1c,
                "b1c": b1c,
                "comb": comb,
                "b2v": b2v,
            }
        )
    return in_maps


def _gather(results):
    out = np.empty((B, H, S, S), np.float32)
    for k in range(N_CORES):
        b = k // (N_CORES // B)
        i0 = (k % (N_CORES // B)) * I_PER_CORE
        out[b, :, i0 : i0 + I_PER_CORE, :] = results[k]["out"]
    return out


def _run(in_maps, **kwargs):
    if "nc" not in _CACHE:
        _CACHE["nc"] = _build_nc()
    return run_bass_kernel_spmd(
        _CACHE["nc"], in_maps, core_ids=list(range(N_CORES)), **kwargs
    )


def kernel(node_features, w1, b1, w2, b2):
    node_features = np.asarray(node_features, np.float32)
    w1 = np.asarray(w1, np.float32)
    b1 = np.asarray(b1, np.float32)
    w2 = np.asarray(w2, np.float32)
    b2 = np.asarray(b2, np.float32)
    in_maps = _host_prep(node_features, w1, b1, w2, b2)
    res = _run(in_maps)
    return _gather(res.results)


def kernel_traced(node_features, w1, b1, w2, b2, **trace_kwargs):
    """test.py helper: same as kernel() but with NTFF tracing enabled."""
    in_maps = _host_prep(
        np.asarray(node_features, np.float32),
        np.asarray(w1, np.float32),
        np.asarray(b1, np.float32),
        np.asarray(w2, np.float32),
        np.asarray(b2, np.float32),
    )
    res = _run(in_maps, trace=True, **trace_kwargs)
    return _gather(res.results), res
